# revision 1
# baseline (speedup 1.0000x reference)
# kernel.py — Mixtral layer (attention + top-2 MoE) on 8 TRN2 NeuronCores.
# Tensor-parallel: attention heads + MoE ffn dim sharded across cores,
# AllReduce (bf16) after o_proj and after MoE w2.
# Self-contained: hardcodes all shapes; host pre-shards/transposes/casts.
import numpy as np
import ml_dtypes

BF16 = ml_dtypes.bfloat16

HID = 1024
NH = 16
NKV = 4
HD = 64
E = 8
FFN = 2048
EPS = 1e-5
THETA = 10000.0
NCORES = 8
FS = FFN // NCORES  # 256 ffn rows per core per expert


# ----------------------------------------------------------------------------
# Device program
# ----------------------------------------------------------------------------
def build_program(S, mock_cc=False):
    import concourse.bass as bass
    import concourse.mybir as mybir
    import concourse.tile as tile
    from concourse import bacc
    from concourse.bass import ts, ds

    dt = mybir.dt
    f32 = dt.float32
    bf16 = dt.bfloat16
    AF = mybir.ActivationFunctionType
    OP = mybir.AluOpType

    NS = S // 512          # 512-wide token slices
    NT = S // 128          # 128-wide token tiles
    HC = HID // 128        # 8 hidden chunks

    nc = bacc.Bacc("TRN2", target_bir_lowering=False, debug=False,
                   num_devices=NCORES)

    # ---- I/O ----
    xT_in = nc.dram_tensor("xT", [HID, S], bf16, kind="ExternalInput").ap()
    xnat_in = nc.dram_tensor("x_nat", [S, HID], f32, kind="ExternalInput").ap()
    cos2_in = nc.dram_tensor("cos2", [128, S], bf16, kind="ExternalInput").ap()
    sin2_in = nc.dram_tensor("sin2", [128, S], bf16, kind="ExternalInput").ap()
    wqT_in = nc.dram_tensor("wqT", [HID, 128], bf16, kind="ExternalInput").ap()
    wkT_in = nc.dram_tensor("wkT", [HID, 64], bf16, kind="ExternalInput").ap()
    wvT_in = nc.dram_tensor("wvT", [HID, 64], bf16, kind="ExternalInput").ap()
    woT_in = nc.dram_tensor("woT", [128, HID], bf16, kind="ExternalInput").ap()
    gateT_in = nc.dram_tensor("gateT", [HID, E], bf16, kind="ExternalInput").ap()
    w1sT_in = nc.dram_tensor("w1sT", [E, HID, FS], bf16, kind="ExternalInput").ap()
    w3sT_in = nc.dram_tensor("w3sT", [E, HID, FS], bf16, kind="ExternalInput").ap()
    w2sT_in = nc.dram_tensor("w2sT", [E, FS, HID], bf16, kind="ExternalInput").ap()
    out_ext = nc.dram_tensor("out", [S, HID], f32, kind="ExternalOutput").ap()

    xT_re = xT_in.rearrange("(c p) t -> p c t", p=128)

    RG = [list(range(NCORES))]

    with tile.TileContext(nc) as tc:
        # ---------- pool stack: cpool -> dram -> mh -> (attn pools) -> mw ----
        cpool = tc.alloc_tile_pool(name="consts", bufs=1)
        dram = tc.alloc_tile_pool(name="dram", bufs=1, space="DRAM")
        mh = tc.alloc_tile_pool(name="mh", bufs=1)  # h2T + wfT (live into MoE)

        # constants
        ones128_bf = cpool.tile([128, 1], bf16)
        nc.vector.memset(ones128_bf, 1.0)
        onesr_f32 = cpool.tile([1, 128], f32)
        nc.vector.memset(onesr_f32, 1.0)
        onesr_bf = cpool.tile([1, 128], bf16)
        nc.vector.memset(onesr_bf, 1.0)
        # epack: rows 0 and 32 select head0/head1 reciprocal rows
        epack = cpool.tile([64, 128], f32)
        nc.vector.memset(epack, 0.0)
        nc.vector.memset(epack[0:1, 0:64], 1.0)
        nc.vector.memset(epack[32:33, 64:128], 1.0)

        # attention weights
        wq_sb = cpool.tile([128, HC, 128], bf16)
        nc.sync.dma_start(wq_sb, wqT_in.rearrange("(c p) m -> p c m", p=128))
        wk_sb = cpool.tile([128, HC, 64], bf16)
        nc.sync.dma_start(wk_sb, wkT_in.rearrange("(c p) m -> p c m", p=128))
        wv_sb = cpool.tile([128, HC, 64], bf16)
        nc.sync.dma_start(wv_sb, wvT_in.rearrange("(c p) m -> p c m", p=128))
        wo_sb = cpool.tile([128, HID], bf16)
        nc.sync.dma_start(wo_sb, woT_in)
        gate_sb = cpool.tile([128, HC, E], bf16)
        nc.sync.dma_start(gate_sb, gateT_in.rearrange("(c p) m -> p c m", p=128))

        # DRAM bounce buffers for collectives
        delta_dram = dram.tile([HID, S], bf16)
        delta_ar = dram.tile([HID, S], bf16, addr_space="Shared")
        y_dram = dram.tile([HID, S], bf16)
        y_ar = dram.tile([HID, S], bf16, addr_space="Shared")
        wfull_dram = dram.tile([NT, 128, E], f32)
        dar_re = delta_ar.rearrange("(c p) t -> p c t", p=128)

        h2T = mh.tile([128, HC, S], bf16)
        wfT_bf = mh.tile([1, E, S], bf16)

        # transposed rms-norm into dst_sb [128, HC, S] bf16.
        # make_src(c, pool) -> AP [128, S] (bf16); called twice per chunk.
        def rmsnorm_T(dst_sb, make_src, tag):
            with tc.tile_pool(name=f"rms_{tag}", bufs=2) as rp, \
                 tc.tile_pool(name=f"rmsp_{tag}", bufs=1, space="PSUM") as pp:
                ss = []
                for si in range(NS):
                    t = pp.tile([1, 512], f32, tag="ss", bufs=NS, name=f"ss{si}")
                    ss.append(t)
                for c in range(HC):
                    src = make_src(c, rp)
                    sq = rp.tile([128, S], bf16, tag="sq", bufs=2, name="sq")
                    nc.scalar.activation(sq, src, AF.Square)
                    for si in range(NS):
                        nc.tensor.matmul(ss[si], ones128_bf, sq[:, ds(512 * si, 512)],
                                         start=(c == 0), stop=(c == HC - 1))
                sccast = []
                for si in range(NS):
                    u = rp.tile([1, 512], f32, tag="u", name="u")
                    nc.vector.tensor_scalar(u, ss[si], 1.0 / HID, EPS, OP.mult, OP.add)
                    r = rp.tile([1, 512], f32, tag="r", name="r")
                    nc.vector.reciprocal(r, u)
                    sc = rp.tile([1, 512], f32, tag="sc", name="sc")
                    nc.scalar.activation(sc, r, AF.Sqrt)
                    scc = pp.tile([128, 512], f32, tag="sccast", bufs=NS,
                                  name=f"sccast{si}")
                    nc.tensor.matmul(scc, onesr_f32, sc)
                    sccast.append(scc)
                for c in range(HC):
                    src = make_src(c, rp)
                    for si in range(NS):
                        nc.vector.tensor_tensor(dst_sb[:, c, ds(512 * si, 512)],
                                                src[:, ds(512 * si, 512)],
                                                sccast[si], OP.mult)

        # ---------- phase 1+2+3: attention ----------
        attnpool = tc.alloc_tile_pool(name="attnpool", bufs=1)
        h1T = attnpool.tile([128, HC, S], bf16)

        def src_x(c, rp):
            xs = rp.tile([128, S], bf16, tag="xs", bufs=2, name="xs")
            nc.sync.dma_start(xs, xT_re[:, c, :])
            return xs

        rmsnorm_T(h1T, src_x, "ln1")

        cos_sb = attnpool.tile([128, S], bf16)
        nc.sync.dma_start(cos_sb, cos2_in)
        sin_sb = attnpool.tile([128, S], bf16)
        nc.sync.dma_start(sin_sb, sin2_in)

        qT_sb = attnpool.tile([64, 2, S], bf16)
        kT_sb = attnpool.tile([64, S], bf16)
        v_sb = attnpool.tile([128, NT, 65], bf16)
        nc.vector.memset(v_sb[:, :, 64:65], 1.0)

        def rope(dsts, src_ps, si, nrows):
            # src_ps: psum [nrows, 512] (nrows = 128 for q(2 heads), 64 for k)
            # dsts: list of per-64-row-group dst APs [64, 512]
            with tc.tile_pool(name="rope", bufs=2) as rpp:
                sl = ds(512 * si, 512)
                rot = rpp.tile([128, 512], bf16, tag="rot", name="rot")
                for h in range(nrows // 64):
                    b = 64 * h
                    nc.vector.tensor_scalar(rot[b:b + 32, :], src_ps[b + 32:b + 64, :],
                                            -1.0, None, OP.mult)
                    nc.vector.tensor_copy(rot[b + 32:b + 64, :], src_ps[b:b + 32, :])
                t1 = rpp.tile([128, 512], bf16, tag="t1", name="t1")
                nc.vector.tensor_tensor(t1[:nrows, :], src_ps, cos_sb[:nrows, sl], OP.mult)
                t2 = rpp.tile([128, 512], bf16, tag="t2", name="t2")
                nc.vector.tensor_tensor(t2[:nrows, :], rot[:nrows, :], sin_sb[:nrows, sl], OP.mult)
                for h, dst in enumerate(dsts):
                    b = 64 * h
                    nc.vector.tensor_tensor(dst, t1[b:b + 64, :], t2[b:b + 64, :], OP.add)

        with tc.tile_pool(name="qkvp", bufs=1, space="PSUM") as qp:
            # q: [64, 2, S] and k: [64, S]
            for si in range(NS):
                sl = ds(512 * si, 512)
                pq = qp.tile([128, 512], f32, tag="pqk", bufs=3, name=f"pq{si}")
                for c in range(HC):
                    nc.tensor.matmul(pq, wq_sb[:, c, :], h1T[:, c, sl],
                                     start=(c == 0), stop=(c == HC - 1))
                rope([qT_sb[:, 0, sl], qT_sb[:, 1, sl]], pq, si, 128)
                pk = qp.tile([128, 512], f32, tag="pqk", bufs=3, name=f"pk{si}")
                for c in range(HC):
                    nc.tensor.matmul(pk[:64, :], wk_sb[:, c, :], h1T[:, c, sl],
                                     start=(c == 0), stop=(c == HC - 1))
                rope([kT_sb[:, sl]], pk[:64, :], si, 64)
            # v natural: [S, 64] as [128, NT, 65] (col 64 = ones for row-sums)
            for i in range(NT):
                pv = qp.tile([128, 64], f32, tag="pv", bufs=2, name="pv")
                for c in range(HC):
                    nc.tensor.matmul(pv, h1T[:, c, ts(i, 128)], wv_sb[:, c, :],
                                     start=(c == 0), stop=(c == HC - 1))
                nc.scalar.copy(v_sb[:, i, 0:64], pv)

        # attention: scores transposed [k, q]; exp without max-subtract
        with tc.tile_pool(name="atsb", bufs=2) as asb, \
             tc.tile_pool(name="atps", bufs=1, space="PSUM") as aps:
            for si in range(NS):
                sl = ds(512 * si, 512)
                attn_ps = [aps.tile([65, 512], f32, tag="attn", bufs=2, name=f"attn{h}")
                           for h in range(2)]
                njt = 4 * si + 4
                for j in range(njt):
                    for h in range(2):
                        st = aps.tile([128, 512], f32, tag="st", bufs=2, name="st")
                        nc.tensor.matmul(st, kT_sb[:, ts(j, 128)], qT_sb[:, h, sl])
                        ex = asb.tile([128, 512], bf16, tag="ex", bufs=3, name="ex")
                        nc.scalar.activation(ex, st, AF.Exp)
                        if j >= 4 * si:
                            nc.gpsimd.affine_select(
                                ex, ex, pattern=[[1, 512]],
                                compare_op=OP.is_ge, fill=0.0,
                                base=512 * si - 128 * j, channel_multiplier=-1)
                        nc.tensor.matmul(attn_ps[h], v_sb[:, j, :], ex,
                                         start=(j == 0), stop=(j == njt - 1))
                # normalize by 1/l  (l = row 64 of attn_ps)
                rp_sb = asb.tile([64, 512], f32, tag="rp", name="rp_sb")
                nc.vector.memset(rp_sb, 0.0)
                nc.vector.reciprocal(rp_sb[0:1, :], attn_ps[0][64:65, :])
                nc.vector.reciprocal(rp_sb[32:33, :], attn_ps[1][64:65, :])
                rc_ps = aps.tile([128, 512], f32, tag="rc", bufs=2, name="rc_ps")
                nc.tensor.matmul(rc_ps, epack, rp_sb)
                rc_sb = asb.tile([128, 512], f32, tag="rcsb", name="rc_sb")
                nc.scalar.copy(rc_sb, rc_ps)
                at_sb = asb.tile([128, 512], bf16, tag="atsb", name="at_sb")
                nc.vector.tensor_tensor(at_sb[0:64, :], attn_ps[0][0:64, :],
                                        rc_sb[0:64, :], OP.mult)
                nc.vector.tensor_tensor(at_sb[64:128, :], attn_ps[1][0:64, :],
                                        rc_sb[64:128, :], OP.mult)
                # delta = woT.T @ attn
                for m in range(HC):
                    dps = aps.tile([128, 512], f32, tag="dps", bufs=2, name="dps")
                    nc.tensor.matmul(dps, wo_sb[:, ts(m, 128)], at_sb)
                    dsb = asb.tile([128, 512], bf16, tag="dsb", name="dsb")
                    nc.vector.tensor_copy(dsb, dps)
                    nc.sync.dma_start(delta_dram[ts(m, 128), sl], dsb)
        attnpool.release()

        # ---------- AR1 ----------
        if mock_cc:
            nc.sync.dma_start(delta_ar, delta_dram)
        else:
            nc.gpsimd.collective_compute("AllReduce", OP.add, replica_groups=RG,
                                         ins=[delta_dram.opt()], outs=[delta_ar.opt()])

        # ---------- rmsnorm2 (x2 streamed from xT + delta_ar) + gate ----------
        def src_x2(c, rp):
            xs = rp.tile([128, S], bf16, tag="xs2", bufs=2, name="xs")
            nc.sync.dma_start(xs, xT_re[:, c, :])
            dr = rp.tile([128, S], bf16, tag="dr", bufs=2, name="dr")
            nc.sync.dma_start(dr, dar_re[:, c, :])
            x2 = rp.tile([128, S], bf16, tag="x2", bufs=2, name="x2")
            nc.vector.tensor_tensor(x2, xs, dr, OP.add)
            return x2

        rmsnorm_T(h2T, src_x2, "ln2")

        with tc.tile_pool(name="gate", bufs=2) as gp, \
             tc.tile_pool(name="gatep", bufs=1, space="PSUM") as gpp:
            for i in range(NT):
                lg = gpp.tile([128, E], f32, tag="lg", bufs=2, name="lg")
                for c in range(HC):
                    nc.tensor.matmul(lg, h2T[:, c, ts(i, 128)], gate_sb[:, c, :],
                                     start=(c == 0), stop=(c == HC - 1))
                lgs = gp.tile([128, E], f32, tag="lgs", name="lgs")
                nc.scalar.copy(lgs, lg)
                top = gp.tile([128, 8], f32, tag="top", name="top")
                nc.vector.max(out=top, in_=lgs)
                dd = gp.tile([128, 1], f32, tag="dd", name="dd")
                nc.vector.tensor_sub(dd, top[:, 0:1], top[:, 1:2])
                w1t = gp.tile([128, 1], f32, tag="w1t", name="w1t")
                nc.scalar.activation(w1t, dd, AF.Sigmoid)
                w2t = gp.tile([128, 1], f32, tag="w2t", name="w2t")
                nc.vector.tensor_scalar(w2t, w1t, -1.0, 1.0, OP.mult, OP.add)
                eq1 = gp.tile([128, E], f32, tag="eq1", name="eq1")
                nc.vector.tensor_scalar(eq1, lgs, top[:, 0:1], None, OP.is_equal)
                eq2 = gp.tile([128, E], f32, tag="eq2", name="eq2")
                nc.vector.tensor_scalar(eq2, lgs, top[:, 1:2], None, OP.is_equal)
                wf1 = gp.tile([128, E], f32, tag="wf1", name="wf1")
                nc.vector.tensor_scalar(wf1, eq1, w1t[:, 0:1], None, OP.mult)
                wfull = gp.tile([128, E], f32, tag="wfull", name="wfull")
                nc.vector.scalar_tensor_tensor(wfull, eq2, w2t[:, 0:1], wf1,
                                               OP.mult, OP.add)
                nc.sync.dma_start(wfull_dram[i], wfull)
            # strided read-back, cast to bf16: [1, E, S]
            wfall = wfull_dram.rearrange("i p e -> e (i p)")
            for e in range(E):
                nc.gpsimd.dma_start(wfT_bf[0:1, e, :], wfall[e:e + 1, :])

        # ---------- MoE (dense over experts; non-selected weight = 0) -------
        mw = tc.alloc_tile_pool(name="mw", bufs=1)
        w2_sb = mw.tile([128, E, 2, HID], bf16)
        nc.sync.dma_start(w2_sb, w2sT_in.rearrange("e (ct p) m -> p e ct m", p=128))
        g_all = mw.tile([128, 2 * E, S], bf16)

        with tc.tile_pool(name="moesb", bufs=2) as msb, \
             tc.tile_pool(name="moeps", bufs=1, space="PSUM") as mps:
            # phase A: g for every expert (w1/w3 streamed per expert)
            for e in range(E):
                w1e = msb.tile([128, HC, FS], bf16, tag="w1e", bufs=2, name="w1e")
                nc.sync.dma_start(w1e, w1sT_in[e].rearrange("(c p) f -> p c f", p=128))
                w3e = msb.tile([128, HC, FS], bf16, tag="w3e", bufs=2, name="w3e")
                nc.sync.dma_start(w3e, w3sT_in[e].rearrange("(c p) f -> p c f", p=128))
                for si in range(NS):
                    sl = ds(512 * si, 512)
                    p13 = []
                    for w_sb, nm in ((w1e, "p1"), (w3e, "p3")):
                        for mt in range(2):
                            p = mps.tile([128, 512], f32, tag="p13", bufs=4,
                                         name=f"{nm}_{mt}")
                            for c in range(HC):
                                nc.tensor.matmul(p, w_sb[:, c, ts(mt, 128)],
                                                 h2T[:, c, sl],
                                                 start=(c == 0), stop=(c == HC - 1))
                            p13.append(p)
                    wc_ps = mps.tile([128, 512], f32, tag="wc", bufs=2, name="wc_ps")
                    nc.tensor.matmul(wc_ps, onesr_bf, wfT_bf[0:1, e, sl])
                    wc_sb = msb.tile([128, 512], bf16, tag="wcsb", name="wc_sb")
                    nc.scalar.copy(wc_sb, wc_ps)
                    for mt in range(2):
                        s1 = msb.tile([128, 512], bf16, tag="s1", name="s1")
                        nc.scalar.activation(s1, p13[mt], AF.Sigmoid)
                        t1 = msb.tile([128, 512], bf16, tag="t1m", name="t1")
                        nc.vector.tensor_tensor(t1, s1, p13[mt], OP.mult)
                        t2 = msb.tile([128, 512], bf16, tag="t2m", name="t2")
                        nc.vector.tensor_tensor(t2, t1, p13[2 + mt], OP.mult)
                        nc.gpsimd.tensor_tensor(g_all[:, 2 * e + mt, sl], t2, wc_sb,
                                                OP.mult)
            # phase B: y = sum_e w2sT_e.T @ g_e  (psum-accumulated over experts)
            for si in range(NS):
                sl = ds(512 * si, 512)
                for m in range(HC):
                    y_ps = mps.tile([128, 512], f32, tag="y", bufs=2, name="y_ps")
                    for e in range(E):
                        for ct in range(2):
                            nc.tensor.matmul(y_ps, w2_sb[:, e, ct, ts(m, 128)],
                                             g_all[:, 2 * e + ct, sl],
                                             start=(e == 0 and ct == 0),
                                             stop=(e == E - 1 and ct == 1))
                    y_sb = msb.tile([128, 512], bf16, tag="ysb", name="y_sb")
                    nc.vector.tensor_copy(y_sb, y_ps)
                    nc.sync.dma_start(y_dram[ts(m, 128), sl], y_sb)

        # ---------- AR2 ----------
        if mock_cc:
            nc.sync.dma_start(y_ar, y_dram)
        else:
            nc.gpsimd.collective_compute("AllReduce", OP.add, replica_groups=RG,
                                         ins=[y_dram.opt()], outs=[y_ar.opt()])
        mw.release()

        # ---------- final: out = x + (delta_ar + y_ar).T ----------
        yar_re = y_ar.rearrange("(c p) t -> p c t", p=128)
        with tc.tile_pool(name="fin", bufs=2) as fp:
            sum_nat = fp.tile([128, NT, HC, 128], bf16, bufs=1, name="sum_nat")
            for c in range(HC):
                ya = fp.tile([128, S], bf16, tag="ya", name="ya")
                nc.sync.dma_start(ya, yar_re[:, c, :])
                da = fp.tile([128, S], bf16, tag="da", name="da")
                nc.sync.dma_start(da, dar_re[:, c, :])
                sm = fp.tile([128, S], bf16, tag="sm", name="sm")
                nc.vector.tensor_tensor(sm, ya, da, OP.add)
                for i in range(NT):
                    nc.sync.dma_start(sum_nat[:, i, c, :], sm[:, ts(i, 128)],
                                      transpose=True)
            for i in range(NT):
                xn = fp.tile([128, HID], f32, tag="xn", name="xn")
                nc.sync.dma_start(xn, xnat_in[ts(i, 128), :])
                ob = fp.tile([128, HID], f32, tag="ob", name="ob")
                nc.vector.tensor_tensor(ob, xn, sum_nat[:, i, :, :], OP.add)
                nc.sync.dma_start(out_ext[ts(i, 128), :], ob)

        mh.release()
        dram.release()
        cpool.release()
    nc.compile()
    return nc


# ----------------------------------------------------------------------------
# Host-side sharding / prep
# ----------------------------------------------------------------------------
def make_in_maps(x, ln1_w, ln2_w, wqkv, wo, gate_w, w13, w2):
    S = x.shape[1]
    x2d = np.asarray(x, np.float32).reshape(S, HID)
    ln1 = np.asarray(ln1_w, np.float32)
    ln2 = np.asarray(ln2_w, np.float32)
    wqkv = np.asarray(wqkv, np.float32)
    wo = np.asarray(wo, np.float32)
    gate_w = np.asarray(gate_w, np.float32)
    w13 = np.asarray(w13, np.float32)
    w2 = np.asarray(w2, np.float32)

    # rope tables
    inv_freq = 1.0 / (THETA ** (np.arange(0, HD, 2, dtype=np.float32) / HD))
    freqs = np.arange(S, dtype=np.float32)[:, None] * inv_freq[None, :]
    emb = np.concatenate([freqs, freqs], axis=-1)  # [S, 64]
    cosT = np.cos(emb).T  # [64, S]
    sinT = np.sin(emb).T
    cos2 = np.ascontiguousarray(np.concatenate([cosT, cosT], 0)).astype(BF16)
    sin2 = np.ascontiguousarray(np.concatenate([sinT, sinT], 0)).astype(BF16)

    xT = np.ascontiguousarray(x2d.T).astype(BF16)      # [HID, S]
    x_nat = np.ascontiguousarray(x2d)                  # [S, HID] f32

    Wq = wqkv[:NH * HD]
    Wk = wqkv[NH * HD:(NH + NKV) * HD]
    Wv = wqkv[(NH + NKV) * HD:]
    gateT = np.ascontiguousarray((gate_w * ln2[None, :]).T).astype(BF16)

    in_maps = []
    for c in range(NCORES):
        g = c // 2
        wq_c = Wq[2 * c * HD:(2 * c + 2) * HD] * ln1[None, :] * (HD ** -0.5)
        wk_c = Wk[g * HD:(g + 1) * HD] * ln1[None, :]
        wv_c = Wv[g * HD:(g + 1) * HD] * ln1[None, :]
        woT_c = wo[:, 2 * c * HD:(2 * c + 2) * HD].T  # [128, HID]
        w1sT = np.stack([
            (w13[e, c * FS:(c + 1) * FS, :] * ln2[None, :]).T for e in range(E)])
        w3sT = np.stack([
            (w13[e, FFN + c * FS:FFN + (c + 1) * FS, :] * ln2[None, :]).T
            for e in range(E)])
        w2sT = np.stack([w2[e][:, c * FS:(c + 1) * FS].T for e in range(E)])
        in_maps.append({
            "xT": xT, "x_nat": x_nat, "cos2": cos2, "sin2": sin2,
            "wqT": np.ascontiguousarray(wq_c.T).astype(BF16),
            "wkT": np.ascontiguousarray(wk_c.T).astype(BF16),
            "wvT": np.ascontiguousarray(wv_c.T).astype(BF16),
            "woT": np.ascontiguousarray(woT_c).astype(BF16),
            "gateT": gateT,
            "w1sT": np.ascontiguousarray(w1sT).astype(BF16),
            "w3sT": np.ascontiguousarray(w3sT).astype(BF16),
            "w2sT": np.ascontiguousarray(w2sT).astype(BF16),
        })
    return in_maps


_CACHED = {}


def kernel(x, ln1_w, ln2_w, wqkv, wo, gate_w, w13, w2):
    from concourse import bass_utils
    S = x.shape[1]
    in_maps = make_in_maps(x, ln1_w, ln2_w, wqkv, wo, gate_w, w13, w2)
    if S not in _CACHED:
        _CACHED[S] = build_program(S)
    nc = _CACHED[S]
    res = bass_utils.run_bass_kernel_spmd(nc, in_maps, core_ids=list(range(NCORES)))
    out = res.results[0]["out"]
    return np.asarray(out, np.float32).reshape(1, S, HID)


if __name__ == "__main__":
    import reference
    inputs = {k: np.asarray(v) for k, v in reference.setup_inputs().items()}
    expected = np.asarray(reference.reference(**{k: v for k, v in inputs.items()}))
    actual = kernel(**inputs)
    err = np.linalg.norm(actual - expected) / np.linalg.norm(expected)
    print("Relative error:", err)



# revision 12
# speedup vs baseline: 1.0560x; 1.0560x over previous
# kernel.py — Mixtral layer (attention + top-2 MoE) on 8 TRN2 NeuronCores.
# Tensor-parallel: attention heads + MoE ffn dim sharded across cores,
# chunked AllReduce (bf16) after o_proj and after MoE w2, overlapped with
# compute. MoE matmuls run in fp8 (e4m3) DoubleRow mode for 2x PE throughput;
# dequant scales are folded into activation/copy ops. The residual (x + attn
# delta) is folded into the MoE output before AR2 (each core adds x2/8), so
# the final output is just a cast of the AR2 result — no transposes or f32
# residual reads on the critical tail.
# Self-contained: hardcodes all shapes; host pre-shards/transposes/casts.
import numpy as np
import ml_dtypes

BF16 = ml_dtypes.bfloat16
F8 = ml_dtypes.float8_e4m3

HID = 1024
NH = 16
NKV = 4
HD = 64
E = 8
FFN = 2048
EPS = 1e-5
THETA = 10000.0
NCORES = 8
FS = FFN // NCORES  # 256 ffn rows per core per expert

W13_SCALE = 16.0    # host multiplies w1/w3 by this before fp8 cast
W2_SCALE = 16.0     # host multiplies w2 by this before fp8 cast
G_SCALE = 8.0       # device represents g*G_SCALE in fp8
# y_psum = (16 w2)^T (8 g wc) = 128 * y_real
Y_DEQ = 1.0 / (W2_SCALE * G_SCALE)


# ----------------------------------------------------------------------------
# Device program
# ----------------------------------------------------------------------------
def build_program(S, mock_cc=False):
    import concourse.bass as bass
    import concourse.mybir as mybir
    import concourse.tile as tile
    from concourse import bacc
    from concourse.bass import ts, ds, _add_dep_helper

    dt = mybir.dt
    f32 = dt.float32
    bf16 = dt.bfloat16
    fp8 = dt.float8e4
    AF = mybir.ActivationFunctionType
    OP = mybir.AluOpType
    DR = mybir.MatmulPerfMode.DoubleRow

    NS = S // 512          # 512-wide token slices
    NT = S // 128          # 128-wide token tiles
    HC = HID // 128        # 8 hidden chunks
    TPG = NT // NS         # token tiles per AR2 group (4)

    nc = bacc.Bacc("TRN2", target_bir_lowering=False, debug=False,
                   num_devices=NCORES)

    # ---- I/O ----
    xT_in = nc.dram_tensor("xT", [HID, S], bf16, kind="ExternalInput").ap()
    cos2_in = nc.dram_tensor("cos2", [128, S], bf16, kind="ExternalInput").ap()
    sin2_in = nc.dram_tensor("sin2", [128, S], bf16, kind="ExternalInput").ap()
    wqT_in = nc.dram_tensor("wqT", [HID, 128], bf16, kind="ExternalInput").ap()
    wkT_in = nc.dram_tensor("wkT", [HID, 64], bf16, kind="ExternalInput").ap()
    wvT_in = nc.dram_tensor("wvT", [HID, 64], bf16, kind="ExternalInput").ap()
    woT_in = nc.dram_tensor("woT", [128, HID], bf16, kind="ExternalInput").ap()
    gateT_in = nc.dram_tensor("gateT", [HID, E], bf16, kind="ExternalInput").ap()
    sel8_in = nc.dram_tensor("sel8", [8, E * 128], bf16, kind="ExternalInput").ap()
    w1sT_in = nc.dram_tensor("w1sT", [E, HID, FS], fp8, kind="ExternalInput").ap()
    w3sT_in = nc.dram_tensor("w3sT", [E, HID, FS], fp8, kind="ExternalInput").ap()
    w2sT_in = nc.dram_tensor("w2sT", [E, FS, HID], fp8, kind="ExternalInput").ap()
    out_ext = nc.dram_tensor("out", [S, HID], f32, kind="ExternalOutput").ap()

    xT_re = xT_in.rearrange("(c p) t -> p c t", p=128)

    RG = [list(range(NCORES))]
    prev_cc = [None]

    def chained_ar(nc_, ins, outs):
        cc = nc_.gpsimd.collective_compute(
            "AllReduce", mybir.AluOpType.add, replica_groups=RG,
            ins=ins, outs=outs)
        if prev_cc[0] is not None:
            _add_dep_helper(cc.ins, prev_cc[0].ins, sync=True,
                            reason="serialize collectives")
        prev_cc[0] = cc
        return cc

    with tile.TileContext(nc) as tc:
        cpool = tc.alloc_tile_pool(name="consts", bufs=1)
        dram = tc.alloc_tile_pool(name="dram", bufs=1, space="DRAM")

        # constants
        ones128_bf = cpool.tile([128, 1], bf16)
        nc.vector.memset(ones128_bf, 1.0)
        onesr_f32 = cpool.tile([1, 128], f32)
        nc.vector.memset(onesr_f32, 1.0)
        # identity (bf16) for PE transpose of gate weights
        ident_bf = cpool.tile([128, 128], bf16)
        nc.vector.memset(ident_bf, 1.0)
        nc.gpsimd.affine_select(ident_bf, ident_bf, pattern=[[1, 128]],
                                compare_op=OP.is_equal, fill=0.0,
                                base=0, channel_multiplier=-1)
        # sel8: row e has ones in cols [128e, 128e+128) — broadcasts wfT8 row e
        sel8 = cpool.tile([8, E * 128], bf16)
        nc.sync.dma_start(sel8, sel8_in)
        # epack: rows 0 and 32 select head0/head1 reciprocal rows
        epack = cpool.tile([64, 128], f32)
        nc.vector.memset(epack, 0.0)
        nc.vector.memset(epack[0:1, 0:64], 1.0)
        nc.vector.memset(epack[32:33, 64:128], 1.0)

        # attention weights
        wq_sb = cpool.tile([128, HC, 128], bf16)
        nc.sync.dma_start(wq_sb, wqT_in.rearrange("(c p) m -> p c m", p=128))
        wk_sb = cpool.tile([128, HC, 64], bf16)
        nc.sync.dma_start(wk_sb, wkT_in.rearrange("(c p) m -> p c m", p=128))
        wv_sb = cpool.tile([128, HC, 64], bf16)
        nc.sync.dma_start(wv_sb, wvT_in.rearrange("(c p) m -> p c m", p=128))
        wo_sb = cpool.tile([128, HID], bf16)
        nc.sync.dma_start(wo_sb, woT_in)
        gate_sb = cpool.tile([128, HC, E], bf16)
        nc.sync.dma_start(gate_sb, gateT_in.rearrange("(c p) m -> p c m", p=128))

        # DRAM bounce buffers for chunked collectives
        d_dram = [dram.tile([HID, 512], bf16, name=f"dd{si}") for si in range(NS)]
        d_ar = [dram.tile([HID, 512], bf16, addr_space="Shared", name=f"da{si}")
                for si in range(NS)]
        y_dram = [dram.tile([512, HID], bf16, name=f"yd{g}") for g in range(NS)]
        y_ar = [dram.tile([512, HID], bf16, addr_space="Shared", name=f"ya{g}")
                for g in range(NS)]

        # long-lived mid tensors (allocated under everything: released last)
        mh = tc.alloc_tile_pool(name="mh", bufs=1)
        h2T = mh.tile([128, HC, S], bf16)     # bf16 h2 (gate)
        h28 = mh.tile([128, HC, S], fp8)      # fp8 h2 (MoE matmuls)
        wfT8 = mh.tile([8, S], bf16)          # per-expert gate weights [E, S]
        x2n = mh.tile([128, NT, HID], bf16)   # (x + delta)/8, natural layout

        # x resident in SBUF; x2 = x + delta overwrites it in place.
        # Released after rmsnorm2/transposes.
        p_x = tc.alloc_tile_pool(name="p_x", bufs=1)
        xsb = p_x.tile([128, HC, S], bf16)
        nc.sync.dma_start(xsb, xT_re)

        # transposed rms-norm: src_ap [128, HC, S] resident; dsts list of
        # [128, HC, S] tiles all receiving src * rsqrt(mean(src^2) + eps)
        def rmsnorm_T(dsts, src_ap, tag):
            with tc.tile_pool(name=f"rms_{tag}", bufs=2) as rp, \
                 tc.tile_pool(name=f"rmsp_{tag}", bufs=1, space="PSUM") as pp:
                ss = []
                for si in range(NS):
                    t = pp.tile([1, 512], f32, tag="ss", bufs=NS, name=f"ss{si}")
                    ss.append(t)
                for c in range(HC):
                    sq = rp.tile([128, S], bf16, tag="sq", bufs=2, name="sq")
                    nc.scalar.activation(sq, src_ap[:, c, :], AF.Square)
                    for si in range(NS):
                        nc.tensor.matmul(ss[si], ones128_bf, sq[:, ds(512 * si, 512)],
                                         start=(c == 0), stop=(c == HC - 1))
                sccast = []
                for si in range(NS):
                    u = rp.tile([1, 512], f32, tag="u", name="u")
                    nc.vector.tensor_scalar(u, ss[si], 1.0 / HID, EPS, OP.mult, OP.add)
                    r = rp.tile([1, 512], f32, tag="r", name="r")
                    nc.vector.reciprocal(r, u)
                    sc = rp.tile([1, 512], f32, tag="sc", name="sc")
                    nc.scalar.activation(sc, r, AF.Sqrt)
                    scc = pp.tile([128, 512], f32, tag="sccast", bufs=NS,
                                  name=f"sccast{si}")
                    nc.tensor.matmul(scc, onesr_f32, sc)
                    sccast.append(scc)
                for c in range(HC):
                    for si in range(NS):
                        nc.vector.tensor_tensor(dsts[0][:, c, ds(512 * si, 512)],
                                                src_ap[:, c, ds(512 * si, 512)],
                                                sccast[si], OP.mult)
                        for dst in dsts[1:]:
                            nc.gpsimd.tensor_copy(dst[:, c, ds(512 * si, 512)],
                                                  dsts[0][:, c, ds(512 * si, 512)])

        # ---------- attention ----------
        attnpool = tc.alloc_tile_pool(name="attnpool", bufs=1)
        h1T = attnpool.tile([128, HC, S], bf16)
        rmsnorm_T([h1T], xsb, "ln1")

        cos_sb = attnpool.tile([128, S], bf16)
        nc.sync.dma_start(cos_sb, cos2_in)
        sin_sb = attnpool.tile([128, S], bf16)
        nc.sync.dma_start(sin_sb, sin2_in)

        qT_sb = attnpool.tile([64, NS, 2, 512], bf16)
        kT_sb = attnpool.tile([64, S], bf16)
        v_sb = attnpool.tile([128, NT, 65], bf16)
        nc.vector.memset(v_sb[:, :, 64:65], 1.0)

        def rope(dsts, src_ps, si, nrows):
            with tc.tile_pool(name="rope", bufs=2) as rpp:
                sl = ds(512 * si, 512)
                rot = rpp.tile([128, 512], bf16, tag="rot", name="rot")
                for h in range(nrows // 64):
                    b = 64 * h
                    nc.vector.tensor_scalar(rot[b:b + 32, :], src_ps[b + 32:b + 64, :],
                                            -1.0, None, OP.mult)
                    nc.vector.tensor_copy(rot[b + 32:b + 64, :], src_ps[b:b + 32, :])
                t1 = rpp.tile([128, 512], bf16, tag="t1", name="t1")
                nc.vector.tensor_tensor(t1[:nrows, :], src_ps, cos_sb[:nrows, sl], OP.mult)
                t2 = rpp.tile([128, 512], bf16, tag="t2", name="t2")
                nc.vector.tensor_tensor(t2[:nrows, :], rot[:nrows, :], sin_sb[:nrows, sl], OP.mult)
                for h, dst in enumerate(dsts):
                    b = 64 * h
                    nc.vector.tensor_tensor(dst, t1[b:b + 64, :], t2[b:b + 64, :], OP.add)

        with tc.tile_pool(name="qkvp", bufs=1, space="PSUM") as qp:
            for si in range(NS):
                sl = ds(512 * si, 512)
                pq = qp.tile([128, 512], f32, tag="pqk", bufs=3, name=f"pq{si}")
                for c in range(HC):
                    nc.tensor.matmul(pq, wq_sb[:, c, :], h1T[:, c, sl],
                                     start=(c == 0), stop=(c == HC - 1))
                rope([qT_sb[:, si, 0, :], qT_sb[:, si, 1, :]], pq, si, 128)
                pk = qp.tile([128, 512], f32, tag="pqk", bufs=3, name=f"pk{si}")
                for c in range(HC):
                    nc.tensor.matmul(pk[:64, :], wk_sb[:, c, :], h1T[:, c, sl],
                                     start=(c == 0), stop=(c == HC - 1))
                rope([kT_sb[:, sl]], pk[:64, :], si, 64)
            for i in range(NT):
                pv = qp.tile([128, 64], f32, tag="pv", bufs=2, name="pv")
                for c in range(HC):
                    nc.tensor.matmul(pv, h1T[:, c, ts(i, 128)], wv_sb[:, c, :],
                                     start=(c == 0), stop=(c == HC - 1))
                nc.scalar.copy(v_sb[:, i, 0:64], pv)

        # attention: scores transposed [k, q]; exp without max-subtract;
        # both heads packed into one scores matmul / exp. AR1 chunk per si.
        with tc.tile_pool(name="atsb", bufs=2) as asb, \
             tc.tile_pool(name="atps", bufs=1, space="PSUM") as aps:
            for si in range(NS):
                sl = ds(512 * si, 512)
                attn_ps = [aps.tile([65, 512], f32, tag="attn", bufs=2, name=f"attn{h}")
                           for h in range(2)]
                njt = 4 * si + 4
                for j in range(njt):
                    st = aps.tile([128, 2, 512], f32, tag="st", bufs=2, name="st")
                    for h in range(2):
                        nc.tensor.matmul(st[:, h, :], kT_sb[:, ts(j, 128)],
                                         qT_sb[:, si, h, :])
                    ex = asb.tile([128, 2, 512], bf16, tag="ex", bufs=3, name="ex")
                    nc.scalar.activation(ex, st, AF.Exp)
                    if j >= 4 * si:
                        for h in range(2):
                            nc.gpsimd.affine_select(
                                ex[:, h, :], ex[:, h, :], pattern=[[1, 512]],
                                compare_op=OP.is_ge, fill=0.0,
                                base=512 * si - 128 * j, channel_multiplier=-1)
                    for h in range(2):
                        nc.tensor.matmul(attn_ps[h], v_sb[:, j, :], ex[:, h, :],
                                         start=(j == 0), stop=(j == njt - 1))
                rp_sb = asb.tile([64, 512], f32, tag="rp", name="rp_sb")
                nc.vector.memset(rp_sb, 0.0)
                nc.vector.reciprocal(rp_sb[0:1, :], attn_ps[0][64:65, :])
                nc.vector.reciprocal(rp_sb[32:33, :], attn_ps[1][64:65, :])
                rc_ps = aps.tile([128, 512], f32, tag="rc", bufs=1, name="rc_ps")
                nc.tensor.matmul(rc_ps, epack, rp_sb)
                rc_sb = asb.tile([128, 512], f32, tag="rcsb", name="rc_sb")
                nc.scalar.copy(rc_sb, rc_ps)
                at_sb = asb.tile([128, 512], bf16, tag="atsb", name="at_sb")
                nc.vector.tensor_tensor(at_sb[0:64, :], attn_ps[0][0:64, :],
                                        rc_sb[0:64, :], OP.mult)
                nc.vector.tensor_tensor(at_sb[64:128, :], attn_ps[1][0:64, :],
                                        rc_sb[64:128, :], OP.mult)
                # delta = woT.T @ attn -> d_dram[si], then AR1 chunk si
                for m in range(HC):
                    dps = aps.tile([128, 512], f32, tag="dps", bufs=1, name="dps")
                    nc.tensor.matmul(dps, wo_sb[:, ts(m, 128)], at_sb)
                    dsb = asb.tile([128, 512], bf16, tag="dsb", name="dsb")
                    nc.vector.tensor_copy(dsb, dps)
                    nc.sync.dma_start(d_dram[si][ts(m, 128), :], dsb)
                if mock_cc:
                    nc.sync.dma_start(d_ar[si], d_dram[si])
                else:
                    chained_ar(nc, [d_dram[si].opt()], [d_ar[si].opt()])
        attnpool.release()

        # ---------- x2 = x + delta, in place over xsb ------------------------
        with tc.tile_pool(name="x2p", bufs=2) as xp:
            for si in range(NS):
                dre = d_ar[si].rearrange("(c p) t -> p c t", p=128)
                for c in range(HC):
                    dd = xp.tile([128, 512], bf16, tag="dd", bufs=3, name="dd")
                    nc.sync.dma_start(dd, dre[:, c, :])
                    nc.vector.tensor_tensor(xsb[:, c, ds(512 * si, 512)],
                                            xsb[:, c, ds(512 * si, 512)], dd, OP.add)
        x2_sb = xsb

        # x2 natural-layout copy (DMA transposes; overlap rmsnorm2 below)
        for c in range(HC):
            for i in range(NT):
                nc.sync.dma_start(x2n[:, i, ts(c, 128)],
                                  x2_sb[:, c, ts(i, 128)], transpose=True)

        rmsnorm_T([h2T, h28], x2_sb, "ln2")

        # x2n *= 1/NCORES (in place) so AR2 sums to y_total + x2
        for i in range(NT):
            nc.vector.tensor_scalar(x2n[:, i, :], x2n[:, i, :],
                                    1.0 / NCORES, None, OP.mult)
        p_x.release()

        # MoE weights resident in fp8 (loads overlap gate below)
        p_moe = tc.alloc_tile_pool(name="p_moe", bufs=1)
        w1_sb = p_moe.tile([128, E, HC, FS], fp8)
        nc.sync.dma_start(w1_sb, w1sT_in.rearrange("e (c p) f -> p e c f", p=128))
        w3_sb = p_moe.tile([128, E, HC, FS], fp8)
        nc.sync.dma_start(w3_sb, w3sT_in.rearrange("e (c p) f -> p e c f", p=128))
        w2_sb = p_moe.tile([128, E, 2, HID], fp8)
        nc.sync.dma_start(w2_sb, w2sT_in.rearrange("e (ct p) m -> p e ct m", p=128))
        g_all = p_moe.tile([128, 2 * E, S], fp8)

        # ---------- gate: top-2 weights -> wfT8 [E, S] via PE transpose ------
        with tc.tile_pool(name="gate", bufs=2) as gp, \
             tc.tile_pool(name="gatep", bufs=1, space="PSUM") as gpp:
            for i in range(NT):
                lg = gpp.tile([128, E], f32, tag="lg", bufs=2, name="lg")
                for c in range(HC):
                    nc.tensor.matmul(lg, h2T[:, c, ts(i, 128)], gate_sb[:, c, :],
                                     start=(c == 0), stop=(c == HC - 1))
                lgs = gp.tile([128, E], f32, tag="lgs", name="lgs")
                nc.scalar.copy(lgs, lg)
                top = gp.tile([128, 8], f32, tag="top", name="top")
                nc.vector.max(out=top, in_=lgs)
                dd = gp.tile([128, 1], f32, tag="dd", name="dd")
                nc.vector.tensor_sub(dd, top[:, 0:1], top[:, 1:2])
                w1t = gp.tile([128, 1], f32, tag="w1t", name="w1t")
                nc.scalar.activation(w1t, dd, AF.Sigmoid)
                w2t = gp.tile([128, 1], f32, tag="w2t", name="w2t")
                nc.vector.tensor_scalar(w2t, w1t, -1.0, 1.0, OP.mult, OP.add)
                eq1 = gp.tile([128, E], f32, tag="eq1", name="eq1")
                nc.vector.tensor_scalar(eq1, lgs, top[:, 0:1], None, OP.is_equal)
                eq2 = gp.tile([128, E], f32, tag="eq2", name="eq2")
                nc.vector.tensor_scalar(eq2, lgs, top[:, 1:2], None, OP.is_equal)
                wf1 = gp.tile([128, E], f32, tag="wf1", name="wf1")
                nc.vector.tensor_scalar(wf1, eq1, w1t[:, 0:1], None, OP.mult)
                wfull = gp.tile([128, E], bf16, tag="wfull", name="wfull")
                nc.vector.scalar_tensor_tensor(wfull, eq2, w2t[:, 0:1], wf1,
                                               OP.mult, OP.add)
                tp_ps = gpp.tile([8, 128], f32, tag="tp", bufs=2, name="tp")
                nc.tensor.matmul(tp_ps, wfull, ident_bf)
                nc.scalar.copy(wfT8[:, ts(i, 128)], tp_ps)

        # ---------- MoE: fp8 DoubleRow, si-major with interleaved phase B ----
        with tc.tile_pool(name="moesb", bufs=2) as msb, \
             tc.tile_pool(name="moeps", bufs=1, space="PSUM") as mps, \
             tc.tile_pool(name="finsb", bufs=2) as fsb:
            for si in range(NS):
                sl = ds(512 * si, 512)
                # phase A: g for every expert at this token slice
                for e in range(E):
                    wc_ps = mps.tile([128, 512], f32, tag="wc", bufs=1, name="wc_ps")
                    nc.tensor.matmul(wc_ps, sel8[:, ts(e, 128)], wfT8[:, sl])
                    wc_sb = msb.tile([128, 512], bf16, tag="wcsb", name="wc_sb")
                    nc.scalar.copy(wc_sb, wc_ps)
                    p13 = {}
                    for w_sb, nm in ((w1_sb, "p1"), (w3_sb, "p3")):
                        for mt in range(2):
                            p = mps.tile([128, 512], f32, tag="p13", bufs=4,
                                         name=f"{nm}_{mt}")
                            for cc in range(0, HC, 2):
                                nc.tensor.matmul(p, w_sb[:, e, cc:cc + 2, ts(mt, 128)],
                                                 h28[:, cc:cc + 2, sl],
                                                 start=(cc == 0), stop=(cc == HC - 2),
                                                 perf_mode=DR)
                            p13[(nm, mt)] = p
                    for mt in range(2):
                        pA = p13[("p1", mt)]
                        pB = p13[("p3", mt)]
                        s1 = msb.tile([128, 512], bf16, tag="s1", name="s1")
                        nc.scalar.activation(s1, pA, AF.Sigmoid, scale=1.0 / W13_SCALE)
                        t1 = msb.tile([128, 512], bf16, tag="t1m", name="t1")
                        nc.vector.tensor_tensor(t1, s1, pA, OP.mult)
                        t2 = msb.tile([128, 512], bf16, tag="t2m", name="t2")
                        nc.vector.scalar_tensor_tensor(
                            t2, pB, G_SCALE / (W13_SCALE * W13_SCALE), t1,
                            OP.mult, OP.mult)
                        nc.gpsimd.tensor_tensor(g_all[:, 2 * e + mt, sl], t2, wc_sb,
                                                OP.mult)
                # phase B: y tiles for this group, + x2/8, -> y_dram[si]
                for it in range(TPG):
                    i = TPG * si + it
                    y_sb = msb.tile([128, HID], bf16, tag="ysb", name="y_sb")
                    for mh2 in range(2):
                        y_ps = mps.tile([128, 512], f32, tag="y", bufs=2,
                                        name=f"y_ps{mh2}")
                        for e in range(E):
                            nc.tensor.matmul(
                                y_ps, g_all[:, 2 * e:2 * e + 2, ts(i, 128)],
                                w2_sb[:, e, :, ts(mh2, 512)],
                                start=(e == 0), stop=(e == E - 1), perf_mode=DR)
                        nc.vector.scalar_tensor_tensor(
                            y_sb[:, ts(mh2, 512)], y_ps, Y_DEQ,
                            x2n[:, i, ts(mh2, 512)], OP.mult, OP.add)
                    nc.sync.dma_start(y_dram[si][ts(it, 128), :], y_sb)
                # AR2 chunk for this group (overlaps next group's compute)
                if mock_cc:
                    nc.sync.dma_start(y_ar[si], y_dram[si])
                else:
                    chained_ar(nc, [y_dram[si].opt()], [y_ar[si].opt()])
                # final: out tiles for this group (cast bf16 -> f32)
                for it in range(TPG):
                    i = TPG * si + it
                    yb = fsb.tile([128, HID], bf16, tag="yb", name="yb")
                    nc.sync.dma_start(yb, y_ar[si][ts(it, 128), :])
                    ob = fsb.tile([128, HID], f32, tag="ob", name="ob")
                    nc.vector.tensor_copy(ob, yb)
                    nc.sync.dma_start(out_ext[ts(i, 128), :], ob)

        p_moe.release()
        mh.release()
        dram.release()
        cpool.release()
    nc.compile()
    return nc


# ----------------------------------------------------------------------------
# Host-side sharding / prep
# ----------------------------------------------------------------------------
def make_in_maps(x, ln1_w, ln2_w, wqkv, wo, gate_w, w13, w2):
    S = x.shape[1]
    x2d = np.asarray(x, np.float32).reshape(S, HID)
    ln1 = np.asarray(ln1_w, np.float32)
    ln2 = np.asarray(ln2_w, np.float32)
    wqkv = np.asarray(wqkv, np.float32)
    wo = np.asarray(wo, np.float32)
    gate_w = np.asarray(gate_w, np.float32)
    w13 = np.asarray(w13, np.float32)
    w2 = np.asarray(w2, np.float32)

    # rope tables
    inv_freq = 1.0 / (THETA ** (np.arange(0, HD, 2, dtype=np.float32) / HD))
    freqs = np.arange(S, dtype=np.float32)[:, None] * inv_freq[None, :]
    emb = np.concatenate([freqs, freqs], axis=-1)  # [S, 64]
    cosT = np.cos(emb).T  # [64, S]
    sinT = np.sin(emb).T
    cos2 = np.ascontiguousarray(np.concatenate([cosT, cosT], 0)).astype(BF16)
    sin2 = np.ascontiguousarray(np.concatenate([sinT, sinT], 0)).astype(BF16)

    xT = np.ascontiguousarray(x2d.T).astype(BF16)      # [HID, S]

    Wq = wqkv[:NH * HD]
    Wk = wqkv[NH * HD:(NH + NKV) * HD]
    Wv = wqkv[(NH + NKV) * HD:]
    gateT = np.ascontiguousarray((gate_w * ln2[None, :]).T).astype(BF16)
    sel8 = np.zeros((8, E * 128), np.float32)
    for e in range(E):
        sel8[e, 128 * e:128 * (e + 1)] = 1.0
    sel8 = sel8.astype(BF16)

    in_maps = []
    for c in range(NCORES):
        g = c // 2
        wq_c = Wq[2 * c * HD:(2 * c + 2) * HD] * ln1[None, :] * (HD ** -0.5)
        wk_c = Wk[g * HD:(g + 1) * HD] * ln1[None, :]
        wv_c = Wv[g * HD:(g + 1) * HD] * ln1[None, :]
        woT_c = wo[:, 2 * c * HD:(2 * c + 2) * HD].T  # [128, HID]
        w1sT = np.stack([
            (W13_SCALE * w13[e, c * FS:(c + 1) * FS, :] * ln2[None, :]).T
            for e in range(E)])
        w3sT = np.stack([
            (W13_SCALE * w13[e, FFN + c * FS:FFN + (c + 1) * FS, :] * ln2[None, :]).T
            for e in range(E)])
        w2sT = np.stack([W2_SCALE * w2[e][:, c * FS:(c + 1) * FS].T
                         for e in range(E)])
        in_maps.append({
            "xT": xT, "cos2": cos2, "sin2": sin2,
            "wqT": np.ascontiguousarray(wq_c.T).astype(BF16),
            "wkT": np.ascontiguousarray(wk_c.T).astype(BF16),
            "wvT": np.ascontiguousarray(wv_c.T).astype(BF16),
            "woT": np.ascontiguousarray(woT_c).astype(BF16),
            "gateT": gateT, "sel8": sel8,
            "w1sT": np.ascontiguousarray(w1sT).astype(F8),
            "w3sT": np.ascontiguousarray(w3sT).astype(F8),
            "w2sT": np.ascontiguousarray(w2sT).astype(F8),
        })
    return in_maps


_CACHED = {}


def kernel(x, ln1_w, ln2_w, wqkv, wo, gate_w, w13, w2):
    from concourse import bass_utils
    S = x.shape[1]
    in_maps = make_in_maps(x, ln1_w, ln2_w, wqkv, wo, gate_w, w13, w2)
    if S not in _CACHED:
        _CACHED[S] = build_program(S)
    nc = _CACHED[S]
    res = bass_utils.run_bass_kernel_spmd(nc, in_maps, core_ids=list(range(NCORES)))
    out = res.results[0]["out"]
    return np.asarray(out, np.float32).reshape(1, S, HID)


if __name__ == "__main__":
    import reference
    inputs = {k: np.asarray(v) for k, v in reference.setup_inputs().items()}
    expected = np.asarray(reference.reference(**{k: v for k, v in inputs.items()}))
    actual = kernel(**inputs)
    err = np.linalg.norm(actual - expected) / np.linalg.norm(expected)
    print("Relative error:", err)
